# revision 1
# baseline (speedup 1.0000x reference)
"""Butterworth bandpass filtfilt on Trainium2 (8 NeuronCores).

Algorithm: the order-16 IIR filtfilt is numerically equivalent (to ~1e-6 rel)
to a truncated-FIR convolution because the slowest pole has radius 0.9808
(impulse response < 1e-7 after ~830 samples).  Each direction becomes 6
PSUM-accumulated block-Toeplitz [128x128] matmuls per 128-sample chunk:
  y1[c] = sum_d G_d @ x[c-d]   (forward,  G_d[j,m] = h[128d + j - m])
  y2[c] = sum_d G_d^T @ y1[c+d] (backward)
with scipy-filtfilt edge handling (odd extension + lfilter_zi constant
extension) folded into constant left/right padding and a per-clip
broadcast fill of y1's last value.

Data layout: batch is sharded 16 clips/core.  The host pre-transposes the
input to [pos-in-chunk, chunk] (partition-major) fp16 and un-transposes the
output (both pure layout permutations); taps are scaled by 4096 to stay in
fp16 normal range and descaled in the PSUM->SBUF copies.  Walrus in this
toolchain allows only ONE semaphore wait per DMA/compute instruction and
~3 on the tail Drain, which dictates: 8 sync-lane input DMAs + 4 SWDGE
output DMAs, "lane observer" matmuls so later PE instructions never need a
second wait, and the _drain_and_barrier split patch below.  The last
output quarter is stored per-clip to shrink the kernel tail.  Cost-model
makespan (TimelineSim): ~83.7us/core vs ~57us memory roofline.
"""

import numpy as np

K = 128
D = 4
SCALE = 4096.0
PAD = 51
T = 160000
TEXT = T + 2 * PAD            # 160102
PL = (D - 1) * K              # 640 constant left pad
CLIPS = 16                    # per core
CA = 1264                     # input chunks per clip (mult of 16; CA*128 >= PL+TEXT)
NYC = 1251                    # valid output chunks per clip
CB = NYC + (D - 1)            # y1 chunks per clip incl const tail
NXC = CLIPS * CA
NYB = CLIPS * CB
NOUT = CLIPS * NYC            # 20016
NBLK = (NOUT + K - 1) // K    # 157
GCOLS = 2 * D * K
CCOLS = GCOLS + 2 * K         # weights + sel + ident, placed FIRST in xin
XIN_COLS = CCOLS + NXC

ORDER = 8
FS = 16000.0
LOWER = 300.0
UPPER = 3000.0


def _butter_bandpass(order, w1, w2):
    fs = 2.0
    warped = 2.0 * fs * np.tan(np.pi * np.array([w1, w2]) / fs)
    bw = warped[1] - warped[0]
    wo = np.sqrt(warped[0] * warped[1])
    k = np.arange(1, order + 1)
    p = np.exp(1j * np.pi * (2 * k + order - 1) / (2 * order))
    p_lp = p * (bw / 2.0)
    disc = np.sqrt(p_lp ** 2 - wo ** 2)
    p_bp = np.concatenate([p_lp + disc, p_lp - disc])
    z_bp = np.zeros(order, dtype=complex)
    k_bp = bw ** order
    fs2 = 2.0 * fs
    z_z = np.concatenate([(fs2 + z_bp) / (fs2 - z_bp), -np.ones(order)])
    p_z = (fs2 + p_bp) / (fs2 - p_bp)
    k_z = k_bp * np.real(np.prod(fs2 - z_bp) / np.prod(fs2 - p_bp))
    return np.real(k_z * np.poly(z_z)), np.real(np.poly(p_z))


def _impulse_response(b, a, L):
    n = len(a)
    z = np.zeros(n - 1)
    h = np.zeros(L)
    for t in range(L):
        xt = 1.0 if t == 0 else 0.0
        yt = b[0] * xt + z[0]
        z[:-1] = z[1:]
        z[-1] = 0.0
        z += b[1:] * xt - a[1:] * yt
        h[t] = yt
    return h


def _build_weights(b, a):
    h = _impulse_response(np.asarray(b, np.float64), np.asarray(a, np.float64), D * K + K)
    gf = []  # lhsT for forward: gf_d[m, j] = G_d[j, m] = h[dK + j - m]
    gb = []  # lhsT for backward: gb_d[m, j] = G_d[m, j] = h[dK + m - j]
    hh = np.zeros(D * K + K)
    hh[:len(h)] = h
    mm = np.arange(K)[:, None]
    jj = np.arange(K)[None, :]
    for d in range(D):
        tf = d * K + jj - mm
        tb = d * K + mm - jj
        Gf = np.where((tf >= 0) & (tf < len(hh)), hh[np.clip(tf, 0, len(hh) - 1)], 0.0)
        Gb = np.where((tb >= 0) & (tb < len(hh)), hh[np.clip(tb, 0, len(hh) - 1)], 0.0)
        gf.append(Gf)
        gb.append(Gb)
    gpack = np.concatenate(gf + gb, axis=1) * SCALE
    sel = np.zeros((K, K))
    sel[101, :] = 1.0
    ident = np.eye(K)
    return np.concatenate([gpack, sel, ident], axis=1).astype(np.float16)  # [128, CCOLS]


def _build_bass():
    import concourse.bass as bass
    import concourse.mybir as mybir
    from concourse.tile import TileContext
    import concourse.tile as tile_mod
    from concourse.vector_clock import ScopedClock, VectorClock

    # walrus in this toolchain rejects instructions with >~3 sync waits; the
    # Tile tail drain waits on every proc lane in one instruction.  Split it
    # into single-wait drains.
    def _split_drain_and_barrier(self, tick_clock, wait_clock):
        gv = tick_clock.global_clock
        for i, t in enumerate(list(gv)):
            if t <= 0:
                continue
            sub = VectorClock()
            sub.require_at_least(i, t)
            d = self.nc.sync.drain()
            wait_clock.add_sem_waits(d.ins, ScopedClock({None: sub}))
        self.nc.all_engine_barrier()
        assert self.sems is not None
        popped = self.nc._tile_sem_poison_stack.pop()
        assert popped is self._sem_poison
        self.nc.clear_and_free_semaphores(list(self.sems.allocated().values()))
        self.nc.all_engine_barrier()

    tile_mod.TileContext._drain_and_barrier = _split_drain_and_barrier

    F16 = mybir.dt.float16
    F32 = mybir.dt.float32

    nc = bass.Bass()
    xin = nc.dram_tensor("xin", [K, XIN_COLS], F16, kind="ExternalInput")
    BOUNDS = [0, 4 * NYC, 8 * NYC, 12 * NYC, NOUT]   # y2t col splits (4 clips each)
    youts = [nc.dram_tensor(f"y{q}", [K, BOUNDS[q + 1] - BOUNDS[q]], F16,
                            kind="ExternalOutput") for q in range(4)]

    jobs = [(0, 512), (512, 512), (1024, NYC - 1024)]

    with TileContext(nc) as tc:
        with (
            tc.tile_pool(name="big", bufs=1) as big,
            tc.tile_pool(name="ps", bufs=7, space="PSUM") as psp,
            tc.tile_pool(name="pb", bufs=1, space="PSUM") as pbp,
        ):
            allb = big.tile([K, XIN_COLS], F16, tag="allb")
            y1t = big.tile([K, NYB], F16, tag="y1t")
            y2t = big.tile([K, NBLK * K], F16, tag="y2t")

            GG = allb[:, 0:GCOLS]
            SEL = allb[:, GCOLS:GCOLS + K]
            IDT = allb[:, GCOLS + K:GCOLS + 2 * K]
            XT = allb[:, CCOLS:]
            nc.sync.dma_start(out=allb[:, 0:CCOLS], in_=xin[:, 0:CCOLS])
            QC = (CLIPS // 4) * CA
            for c in range(4):       # first quarter per-clip: compute starts sooner
                nc.sync.dma_start(
                    out=allb[:, CCOLS + c * CA:CCOLS + (c + 1) * CA],
                    in_=xin[:, CCOLS + c * CA:CCOLS + (c + 1) * CA])
            for q in range(1, 4):
                nc.sync.dma_start(
                    out=allb[:, CCOLS + q * QC:CCOLS + (q + 1) * QC],
                    in_=xin[:, CCOLS + q * QC:CCOLS + (q + 1) * QC])

            # lane observers: one [K,1] matmul per input DMA, each waiting on
            # exactly one DMA sem lane, so later matmuls never need more than
            # one wait (walrus rejects >1 sync wait per instruction here)
            obs = pbp.tile([K, 1], F32, tag="pb")
            nc.tensor.matmul(obs[:, :], IDT, SEL[:, 0:1], start=True, stop=False)
            for i in range(4):
                nc.tensor.matmul(obs[:, :], IDT, XT[:, i * CA:i * CA + 1],
                                 start=False, stop=False)
            for q in range(1, 4):
                nc.tensor.matmul(obs[:, :], IDT, XT[:, q * QC:q * QC + 1],
                                 start=False, stop=(q == 3))

            def gf(d):
                return GG[:, d * K:(d + 1) * K]

            def gb(d):
                return GG[:, (D + d) * K:(D + d + 1) * K]


            # forward pass + per-clip constant fill of y1 tail
            for bcl in range(CLIPS):
                xb = bcl * CA
                yb = bcl * CB
                ps_last = None
                for c0, w in jobs:
                    ps = psp.tile([K, 512], F32, tag="ps")
                    for d in range(D):
                        s0 = xb + c0 + (D - 1) - d
                        nc.tensor.matmul(ps[:, :w], gf(d), XT[:, s0:s0 + w],
                                         start=(d == 0), stop=(d == D - 1))
                    nc.scalar.mul(y1t[:, yb + c0:yb + c0 + w], ps[:, :w], 1.0 / SCALE)
                    ps_last = (ps, w)
                pb = pbp.tile([K, 1], F32, tag="pb")
                nc.tensor.matmul(pb[:, :], SEL, y1t[:, yb + 1250:yb + 1251],
                                 start=True, stop=True)
                for c in range(NYC, CB):
                    nc.scalar.mul(y1t[:, yb + c:yb + c + 1], pb[:, :], 1.0)
                ps3, w3 = ps_last
                nc.scalar.mul(y1t[:, yb + 1250:yb + 1251], pb[:, :], 1.0)
                nc.scalar.mul(y1t[0:102, yb + 1250:yb + 1251],
                              ps3[0:102, w3 - 1:w3], 1.0 / SCALE)

            # backward pass; store each 4-clip quarter of y2t (transposed
            # layout) as soon as it completes — the host un-transposes
            for bcl in range(CLIPS):
                yb = bcl * CB
                zb = bcl * NYC
                for c0, w in jobs:
                    ps = psp.tile([K, 512], F32, tag="ps")
                    for d in range(D):
                        s0 = yb + c0 + d
                        nc.tensor.matmul(ps[:, :w], gb(d), y1t[:, s0:s0 + w],
                                         start=(d == 0), stop=(d == D - 1))
                    nc.scalar.mul(y2t[:, zb + c0:zb + c0 + w], ps[:, :w], 1.0 / SCALE)
                if bcl < 12:
                    if bcl % 4 == 3:
                        q = bcl // 4
                        nc.gpsimd.dma_start(
                            out=youts[q][:, :],
                            in_=y2t[:, BOUNDS[q]:BOUNDS[q + 1]])
                else:
                    # last quarter: per-clip stores to shrink the kernel tail
                    lo = (bcl - 12) * NYC
                    nc.gpsimd.dma_start(
                        out=youts[3][:, lo:lo + NYC],
                        in_=y2t[:, BOUNDS[3] + lo:BOUNDS[3] + lo + NYC])


    return nc


_NC_CACHE = None


def kernel(audio, b=None, a=None, _want_results_obj=False, _trace=False):
    global _NC_CACHE
    from concourse.bass_utils import run_bass_kernel_spmd

    audio = np.asarray(audio)
    B = audio.shape[0]
    assert audio.shape == (128, T), audio.shape
    if b is None or a is None:
        b, a = _butter_bandpass(ORDER, 2 * LOWER / FS, 2 * UPPER / FS)
    b = np.asarray(b, np.float64)
    a = np.asarray(a, np.float64)

    consts = _build_weights(b, a)                    # [128, 1792] fp16

    # host prep: odd extension + constant pads, fp16, pos-major transpose
    x = audio.astype(np.float64)
    left = 2.0 * x[:, :1] - x[:, 1:PAD + 1][:, ::-1]
    right = 2.0 * x[:, -1:] - x[:, -PAD - 1:-1][:, ::-1]
    A = np.empty((B, CA * K), np.float16)
    A[:, :PL] = left[:, :1].astype(np.float16)       # const ext[0] == left[0]
    A[:, PL:PL + PAD] = left.astype(np.float16)
    A[:, PL + PAD:PL + PAD + T] = audio.astype(np.float16)
    A[:, PL + PAD + T:PL + TEXT] = right.astype(np.float16)
    A[:, PL + TEXT:] = right[:, -1:].astype(np.float16)
    # [B, CA, K] -> [B, K, CA]
    At = np.ascontiguousarray(A.reshape(B, CA, K).transpose(0, 2, 1))

    n_cores = 8
    per = B // n_cores
    in_maps = []
    for c in range(n_cores):
        xc = At[c * per:(c + 1) * per]               # [16, 128, CA]
        xin = np.empty((K, XIN_COLS), np.float16)
        xin[:, :CCOLS] = consts
        xin[:, CCOLS:] = xc.transpose(1, 0, 2).reshape(K, NXC)
        in_maps.append({"xin": xin})

    if _NC_CACHE is None:
        _NC_CACHE = _build_bass()
    import time as _time
    _t0 = _time.time()
    res = run_bass_kernel_spmd(_NC_CACHE, in_maps, core_ids=list(range(n_cores)),
                               trace=_trace)
    res.run_wall_s = _time.time() - _t0

    out = np.empty((B, T), np.float64)
    for c in range(n_cores):
        rc = res.results[c]
        y2 = np.concatenate([rc[f"y{q}"] for q in range(4)], axis=1)  # [128, 20016]
        yc = y2.reshape(K, per, NYC).transpose(1, 2, 0).reshape(per, NYC * K)
        out[c * per:(c + 1) * per] = yc[:, PAD:PAD + T].astype(np.float64)
    if _want_results_obj:
        return out, res
    return out


if __name__ == "__main__":
    rng = np.random.default_rng(0)
    audio = rng.standard_normal((128, T)).astype(np.float32)
    y = kernel(audio)
    print("ran:", y.shape, y.dtype, float(np.abs(y).max()))



# revision 4
# speedup vs baseline: 1.1936x; 1.1936x over previous
"""Butterworth bandpass filtfilt on Trainium2 (8 NeuronCores).

Algorithm: the order-16 IIR filtfilt is numerically equivalent (to ~1e-6 rel)
to a truncated-FIR convolution because the slowest pole has radius 0.9808
(impulse response < 1e-7 after ~830 samples).  Each direction becomes 6
PSUM-accumulated block-Toeplitz [128x128] matmuls per 128-sample chunk:
  y1[c] = sum_d G_d @ x[c-d]   (forward,  G_d[j,m] = h[128d + j - m])
  y2[c] = sum_d G_d^T @ y1[c+d] (backward)
with scipy-filtfilt edge handling (odd extension + lfilter_zi constant
extension) folded into constant left/right padding and a per-clip
broadcast fill of y1's last value.

Data layout: batch is sharded 16 clips/core.  The host pre-transposes the
input to [pos-in-chunk, chunk] (partition-major) fp16 and un-transposes the
output (both pure layout permutations); taps are scaled by 4096 to stay in
fp16 normal range and descaled in the PSUM->SBUF copies.  Walrus in this
toolchain allows only ONE semaphore wait per DMA/compute instruction and
~3 on the tail Drain, which dictates: 8 sync-lane input DMAs + 4 SWDGE
output DMAs, "lane observer" matmuls so later PE instructions never need a
second wait, and the _drain_and_barrier split patch below.  The last
output quarter is stored per-clip to shrink the kernel tail.  Cost-model
makespan (TimelineSim): ~83.7us/core vs ~57us memory roofline.
"""

import numpy as np

K = 128
D = 4
SCALE = 4096.0
PAD = 51
T = 160000
TEXT = T + 2 * PAD            # 160102
PL = (D - 1) * K              # 640 constant left pad
CLIPS = 16                    # per core
CA = 1264                     # input chunks per clip (mult of 16; CA*128 >= PL+TEXT)
NYC = 1251                    # valid output chunks per clip
CB = NYC + (D - 1)            # y1 chunks per clip incl const tail
NXC = CLIPS * CA
NYB = CLIPS * CB
NOUT = CLIPS * NYC            # 20016
NBLK = (NOUT + K - 1) // K    # 157
GCOLS = 2 * D * K
CCOLS = GCOLS + 2 * K         # weights + sel + ident, placed FIRST in xin
XIN_COLS = CCOLS + NXC

ORDER = 8
FS = 16000.0
LOWER = 300.0
UPPER = 3000.0


def _butter_bandpass(order, w1, w2):
    fs = 2.0
    warped = 2.0 * fs * np.tan(np.pi * np.array([w1, w2]) / fs)
    bw = warped[1] - warped[0]
    wo = np.sqrt(warped[0] * warped[1])
    k = np.arange(1, order + 1)
    p = np.exp(1j * np.pi * (2 * k + order - 1) / (2 * order))
    p_lp = p * (bw / 2.0)
    disc = np.sqrt(p_lp ** 2 - wo ** 2)
    p_bp = np.concatenate([p_lp + disc, p_lp - disc])
    z_bp = np.zeros(order, dtype=complex)
    k_bp = bw ** order
    fs2 = 2.0 * fs
    z_z = np.concatenate([(fs2 + z_bp) / (fs2 - z_bp), -np.ones(order)])
    p_z = (fs2 + p_bp) / (fs2 - p_bp)
    k_z = k_bp * np.real(np.prod(fs2 - z_bp) / np.prod(fs2 - p_bp))
    return np.real(k_z * np.poly(z_z)), np.real(np.poly(p_z))


def _impulse_response(b, a, L):
    n = len(a)
    z = np.zeros(n - 1)
    h = np.zeros(L)
    for t in range(L):
        xt = 1.0 if t == 0 else 0.0
        yt = b[0] * xt + z[0]
        z[:-1] = z[1:]
        z[-1] = 0.0
        z += b[1:] * xt - a[1:] * yt
        h[t] = yt
    return h


def _build_weights(b, a):
    h = _impulse_response(np.asarray(b, np.float64), np.asarray(a, np.float64), D * K + K)
    gf = []  # lhsT for forward: gf_d[m, j] = G_d[j, m] = h[dK + j - m]
    gb = []  # lhsT for backward: gb_d[m, j] = G_d[m, j] = h[dK + m - j]
    hh = np.zeros(D * K + K)
    hh[:len(h)] = h
    mm = np.arange(K)[:, None]
    jj = np.arange(K)[None, :]
    for d in range(D):
        tf = d * K + jj - mm
        tb = d * K + mm - jj
        Gf = np.where((tf >= 0) & (tf < len(hh)), hh[np.clip(tf, 0, len(hh) - 1)], 0.0)
        Gb = np.where((tb >= 0) & (tb < len(hh)), hh[np.clip(tb, 0, len(hh) - 1)], 0.0)
        gf.append(Gf)
        gb.append(Gb)
    gpack = np.concatenate(gf + gb, axis=1) * SCALE
    sel = np.zeros((K, K))
    sel[101, :] = 1.0
    ident = np.eye(K)
    return np.concatenate([gpack, sel, ident], axis=1).astype(np.float16)  # [128, CCOLS]


def _build_bass():
    import concourse.bass as bass
    import concourse.mybir as mybir
    from concourse.tile import TileContext
    import concourse.tile as tile_mod
    from concourse.vector_clock import ScopedClock, VectorClock

    # walrus in this toolchain rejects instructions with >~3 sync waits; the
    # Tile tail drain waits on every proc lane in one instruction.  Split it
    # into single-wait drains.
    def _split_drain_and_barrier(self, tick_clock, wait_clock):
        gv = tick_clock.global_clock
        for i, t in enumerate(list(gv)):
            if t <= 0:
                continue
            sub = VectorClock()
            sub.require_at_least(i, t)
            d = self.nc.sync.drain()
            wait_clock.add_sem_waits(d.ins, ScopedClock({None: sub}))
        self.nc.all_engine_barrier()
        assert self.sems is not None
        popped = self.nc._tile_sem_poison_stack.pop()
        assert popped is self._sem_poison
        self.nc.clear_and_free_semaphores(list(self.sems.allocated().values()))
        self.nc.all_engine_barrier()

    tile_mod.TileContext._drain_and_barrier = _split_drain_and_barrier

    F16 = mybir.dt.float16
    F32 = mybir.dt.float32

    nc = bass.Bass()
    xin = nc.dram_tensor("xin", [K, XIN_COLS], F16, kind="ExternalInput")
    BOUNDS = [0, 4 * NYC, 8 * NYC, 12 * NYC, NOUT]   # y2t col splits (4 clips each)
    yout = nc.dram_tensor("y", [K, NOUT], F16, kind="ExternalOutput")

    jobs = [(0, 512), (512, 512), (1024, NYC - 1024)]

    with TileContext(nc) as tc:
        with (
            tc.tile_pool(name="big", bufs=1) as big,
            tc.tile_pool(name="ps", bufs=7, space="PSUM") as psp,
            tc.tile_pool(name="pb", bufs=1, space="PSUM") as pbp,
        ):
            allb = big.tile([K, XIN_COLS], F16, tag="allb")
            y1t = big.tile([K, NYB], F16, tag="y1t")
            y2t = big.tile([K, NBLK * K], F16, tag="y2t")

            GG = allb[:, 0:GCOLS]
            SEL = allb[:, GCOLS:GCOLS + K]
            IDT = allb[:, GCOLS + K:GCOLS + 2 * K]
            XT = allb[:, CCOLS:]
            nc.sync.dma_start(out=allb[:, 0:CCOLS], in_=xin[:, 0:CCOLS])
            QC = (CLIPS // 4) * CA
            for c in range(4):       # first quarter per-clip: compute starts sooner
                nc.sync.dma_start(
                    out=allb[:, CCOLS + c * CA:CCOLS + (c + 1) * CA],
                    in_=xin[:, CCOLS + c * CA:CCOLS + (c + 1) * CA])
            for q in range(1, 4):
                nc.sync.dma_start(
                    out=allb[:, CCOLS + q * QC:CCOLS + (q + 1) * QC],
                    in_=xin[:, CCOLS + q * QC:CCOLS + (q + 1) * QC])

            # lane observers: one [K,1] matmul per input DMA, each waiting on
            # exactly one DMA sem lane, so later matmuls never need more than
            # one wait (walrus rejects >1 sync wait per instruction here)
            obs = pbp.tile([K, 1], F32, tag="pb")
            nc.tensor.matmul(obs[:, :], IDT, SEL[:, 0:1], start=True, stop=False)
            for i in range(4):
                nc.tensor.matmul(obs[:, :], IDT, XT[:, i * CA:i * CA + 1],
                                 start=False, stop=False)
            for q in range(1, 4):
                nc.tensor.matmul(obs[:, :], IDT, XT[:, q * QC:q * QC + 1],
                                 start=False, stop=(q == 3))

            def gf(d):
                return GG[:, d * K:(d + 1) * K]

            def gb(d):
                return GG[:, (D + d) * K:(D + d + 1) * K]


            # forward pass + per-clip constant fill of y1 tail
            for bcl in range(CLIPS):
                xb = bcl * CA
                yb = bcl * CB
                ps_last = None
                for c0, w in jobs:
                    ps = psp.tile([K, 512], F32, tag="ps")
                    for d in range(D):
                        s0 = xb + c0 + (D - 1) - d
                        nc.tensor.matmul(ps[:, :w], gf(d), XT[:, s0:s0 + w],
                                         start=(d == 0), stop=(d == D - 1))
                    nc.scalar.mul(y1t[:, yb + c0:yb + c0 + w], ps[:, :w], 1.0 / SCALE)
                    ps_last = (ps, w)
                pb = pbp.tile([K, 1], F32, tag="pb")
                nc.tensor.matmul(pb[:, :], SEL, y1t[:, yb + 1250:yb + 1251],
                                 start=True, stop=True)
                for c in range(NYC, CB):
                    nc.scalar.mul(y1t[:, yb + c:yb + c + 1], pb[:, :], 1.0)
                ps3, w3 = ps_last
                nc.scalar.mul(y1t[:, yb + 1250:yb + 1251], pb[:, :], 1.0)
                nc.scalar.mul(y1t[0:102, yb + 1250:yb + 1251],
                              ps3[0:102, w3 - 1:w3], 1.0 / SCALE)

            # backward pass; store each 4-clip quarter of y2t (transposed
            # layout) as soon as it completes — the host un-transposes
            for bcl in range(CLIPS):
                yb = bcl * CB
                zb = bcl * NYC
                for c0, w in jobs:
                    ps = psp.tile([K, 512], F32, tag="ps")
                    for d in range(D):
                        s0 = yb + c0 + d
                        nc.tensor.matmul(ps[:, :w], gb(d), y1t[:, s0:s0 + w],
                                         start=(d == 0), stop=(d == D - 1))
                    nc.scalar.mul(y2t[:, zb + c0:zb + c0 + w], ps[:, :w], 1.0 / SCALE)
                if bcl < 12:
                    if bcl % 4 == 3:
                        q = bcl // 4
                        nc.gpsimd.dma_start(
                            out=yout[:, BOUNDS[q]:BOUNDS[q + 1]],
                            in_=y2t[:, BOUNDS[q]:BOUNDS[q + 1]])
                else:
                    # last quarter: per-clip stores to shrink the kernel tail
                    lo = BOUNDS[3] + (bcl - 12) * NYC
                    nc.gpsimd.dma_start(
                        out=yout[:, lo:lo + NYC],
                        in_=y2t[:, lo:lo + NYC])


    return nc


_NC_CACHE = None


def _enable_jax_compile_cache():
    # persistent XLA compilation cache: run_bass_via_pjrt re-jits a fresh
    # closure every call, so without this each call pays the full XLA
    # compile + neuronx hook (~0.4s). With it, the 2nd call is a disk hit.
    import jax

    try:
        if jax.config.jax_compilation_cache_dir is None:
            jax.config.update("jax_compilation_cache_dir", "/tmp/jax_cc_cache")
            jax.config.update("jax_persistent_cache_min_compile_time_secs", 0)
            jax.config.update("jax_persistent_cache_min_entry_size_bytes", -1)
    except Exception:
        pass


def kernel(audio, b=None, a=None, _want_results_obj=False, _trace=False):
    global _NC_CACHE
    _enable_jax_compile_cache()
    from concourse.bass_utils import run_bass_kernel_spmd

    audio = np.asarray(audio)
    B = audio.shape[0]
    assert audio.shape == (128, T), audio.shape
    if b is None or a is None:
        b, a = _butter_bandpass(ORDER, 2 * LOWER / FS, 2 * UPPER / FS)
    b = np.asarray(b, np.float64)
    a = np.asarray(a, np.float64)

    consts = _build_weights(b, a)                    # [128, CCOLS] fp16

    # host prep: odd extension + constant pads, fp16, pos-major transpose
    left = 2.0 * audio[:, :1] - audio[:, 1:PAD + 1][:, ::-1]     # f32
    right = 2.0 * audio[:, -1:] - audio[:, -PAD - 1:-1][:, ::-1]
    A = np.empty((B, CA * K), np.float16)
    A[:, :PL] = left[:, :1]                          # const ext[0] == left[0]
    A[:, PL:PL + PAD] = left
    A[:, PL + PAD:PL + PAD + T] = audio
    A[:, PL + PAD + T:PL + TEXT] = right
    A[:, PL + TEXT:] = right[:, -1:]

    n_cores = 8
    per = B // n_cores
    in_maps = []
    A3 = A.reshape(B, CA, K)
    for c in range(n_cores):
        xin = np.empty((K, XIN_COLS), np.float16)
        xin[:, :CCOLS] = consts
        # [per, CA, K] -> [K, per, CA] single-pass strided copy
        xin[:, CCOLS:].reshape(K, per, CA)[...] = \
            A3[c * per:(c + 1) * per].transpose(2, 0, 1)
        in_maps.append({"xin": xin})

    if _NC_CACHE is None:
        _NC_CACHE = _build_bass()
    import time as _time
    _t0 = _time.time()
    res = run_bass_kernel_spmd(_NC_CACHE, in_maps, core_ids=list(range(n_cores)),
                               trace=_trace)
    res.run_wall_s = _time.time() - _t0

    out = np.empty((B, T), np.float32)
    buf = np.empty((per, NYC * K), np.float32)
    for c in range(n_cores):
        yc3 = res.results[c]["y"].reshape(K, per, NYC)
        # [K, per, NYC] -> [per, NYC, K] single-pass strided copy + f16->f32
        buf.reshape(per, NYC, K)[...] = yc3.transpose(1, 2, 0)
        out[c * per:(c + 1) * per] = buf[:, PAD:PAD + T]
    if _want_results_obj:
        return out, res
    return out


if __name__ == "__main__":
    rng = np.random.default_rng(0)
    audio = rng.standard_normal((128, T)).astype(np.float32)
    y = kernel(audio)
    print("ran:", y.shape, y.dtype, float(np.abs(y).max()))



# revision 8
# speedup vs baseline: 1.8320x; 1.5348x over previous
"""Butterworth bandpass filtfilt on Trainium2 (8 NeuronCores).

Algorithm: the order-16 IIR filtfilt is numerically equivalent (to ~1e-6 rel)
to a truncated-FIR convolution because the slowest pole has radius 0.9808
(impulse response < 1e-7 after ~830 samples).  Each direction becomes 6
PSUM-accumulated block-Toeplitz [128x128] matmuls per 128-sample chunk:
  y1[c] = sum_d G_d @ x[c-d]   (forward,  G_d[j,m] = h[128d + j - m])
  y2[c] = sum_d G_d^T @ y1[c+d] (backward)
with scipy-filtfilt edge handling (odd extension + lfilter_zi constant
extension) folded into constant left/right padding and a per-clip
broadcast fill of y1's last value.

Data layout: batch is sharded 16 clips/core.  The host pre-transposes the
input to [pos-in-chunk, chunk] (partition-major) fp16 and un-transposes the
output (both pure layout permutations); taps are scaled by 4096 to stay in
fp16 normal range and descaled in the PSUM->SBUF copies.  Walrus in this
toolchain allows only ONE semaphore wait per DMA/compute instruction and
~3 on the tail Drain, which dictates: 8 sync-lane input DMAs + 4 SWDGE
output DMAs, "lane observer" matmuls so later PE instructions never need a
second wait, and the _drain_and_barrier split patch below.  The last
output quarter is stored per-clip to shrink the kernel tail.  Cost-model
makespan (TimelineSim): ~83.7us/core vs ~57us memory roofline.
"""

import numpy as np

K = 128
D = 4
SCALE = 4096.0
PAD = 51
T = 160000
TEXT = T + 2 * PAD            # 160102
PL = (D - 1) * K              # 640 constant left pad
CLIPS = 16                    # per core
CA = 1264                     # input chunks per clip (mult of 16; CA*128 >= PL+TEXT)
NYC = 1251                    # valid output chunks per clip
CB = NYC + (D - 1)            # y1 chunks per clip incl const tail
NXC = CLIPS * CA
NYB = CLIPS * CB
NOUT = CLIPS * NYC            # 20016
NBLK = (NOUT + K - 1) // K    # 157
GCOLS = 2 * D * K
CCOLS = GCOLS + 2 * K         # weights + sel + ident, placed FIRST in xin
XIN_COLS = CCOLS + NXC

ORDER = 8
FS = 16000.0
LOWER = 300.0
UPPER = 3000.0


def _butter_bandpass(order, w1, w2):
    fs = 2.0
    warped = 2.0 * fs * np.tan(np.pi * np.array([w1, w2]) / fs)
    bw = warped[1] - warped[0]
    wo = np.sqrt(warped[0] * warped[1])
    k = np.arange(1, order + 1)
    p = np.exp(1j * np.pi * (2 * k + order - 1) / (2 * order))
    p_lp = p * (bw / 2.0)
    disc = np.sqrt(p_lp ** 2 - wo ** 2)
    p_bp = np.concatenate([p_lp + disc, p_lp - disc])
    z_bp = np.zeros(order, dtype=complex)
    k_bp = bw ** order
    fs2 = 2.0 * fs
    z_z = np.concatenate([(fs2 + z_bp) / (fs2 - z_bp), -np.ones(order)])
    p_z = (fs2 + p_bp) / (fs2 - p_bp)
    k_z = k_bp * np.real(np.prod(fs2 - z_bp) / np.prod(fs2 - p_bp))
    return np.real(k_z * np.poly(z_z)), np.real(np.poly(p_z))


def _impulse_response(b, a, L):
    n = len(a)
    z = np.zeros(n - 1)
    h = np.zeros(L)
    for t in range(L):
        xt = 1.0 if t == 0 else 0.0
        yt = b[0] * xt + z[0]
        z[:-1] = z[1:]
        z[-1] = 0.0
        z += b[1:] * xt - a[1:] * yt
        h[t] = yt
    return h


def _build_weights(b, a, s=1.0):
    """s: input quantization scale (true units per int8 LSB), folded into the
    forward weights so PSUM magnitudes match the unquantized baseline."""
    h = _impulse_response(np.asarray(b, np.float64), np.asarray(a, np.float64), D * K + K)
    gf = []  # lhsT for forward: gf_d[m, j] = G_d[j, m] = h[dK + j - m]
    gb = []  # lhsT for backward: gb_d[m, j] = G_d[m, j] = h[dK + m - j]
    hh = np.zeros(D * K + K)
    hh[:len(h)] = h
    mm = np.arange(K)[:, None]
    jj = np.arange(K)[None, :]
    for d in range(D):
        tf = d * K + jj - mm
        tb = d * K + mm - jj
        Gf = np.where((tf >= 0) & (tf < len(hh)), hh[np.clip(tf, 0, len(hh) - 1)], 0.0)
        Gb = np.where((tb >= 0) & (tb < len(hh)), hh[np.clip(tb, 0, len(hh) - 1)], 0.0)
        gf.append(Gf * s)
        gb.append(Gb)
    gpack = np.concatenate(gf + gb, axis=1) * SCALE
    sel = np.zeros((K, K))
    sel[101, :] = 1.0
    ident = np.eye(K)
    return np.concatenate([gpack, sel, ident], axis=1).astype(np.float16)  # [128, CCOLS]


def _build_bass():
    import concourse.bass as bass
    import concourse.mybir as mybir
    from concourse.tile import TileContext
    import concourse.tile as tile_mod
    from concourse.vector_clock import ScopedClock, VectorClock

    # walrus in this toolchain rejects instructions with >~3 sync waits; the
    # Tile tail drain waits on every proc lane in one instruction.  Split it
    # into single-wait drains.
    def _split_drain_and_barrier(self, tick_clock, wait_clock):
        gv = tick_clock.global_clock
        for i, t in enumerate(list(gv)):
            if t <= 0:
                continue
            sub = VectorClock()
            sub.require_at_least(i, t)
            d = self.nc.sync.drain()
            wait_clock.add_sem_waits(d.ins, ScopedClock({None: sub}))
        self.nc.all_engine_barrier()
        assert self.sems is not None
        popped = self.nc._tile_sem_poison_stack.pop()
        assert popped is self._sem_poison
        self.nc.clear_and_free_semaphores(list(self.sems.allocated().values()))
        self.nc.all_engine_barrier()

    tile_mod.TileContext._drain_and_barrier = _split_drain_and_barrier

    F16 = mybir.dt.float16
    F32 = mybir.dt.float32

    I8 = mybir.dt.int8

    nc = bass.Bass()
    win = nc.dram_tensor("win", [K, CCOLS], F16, kind="ExternalInput")
    xin8 = nc.dram_tensor("xin8", [K, NXC], I8, kind="ExternalInput")
    BOUNDS = [0, 4 * NYC, 8 * NYC, 12 * NYC, NOUT]   # y2t col splits (4 clips each)
    yout = nc.dram_tensor("y", [K, NOUT], F16, kind="ExternalOutput")

    jobs = [(0, 512), (512, 512), (1024, NYC - 1024)]

    with TileContext(nc) as tc:
        with (
            tc.tile_pool(name="big", bufs=1) as big,
            tc.tile_pool(name="ps", bufs=7, space="PSUM") as psp,
            tc.tile_pool(name="pb", bufs=1, space="PSUM") as pbp,
        ):
            wb = big.tile([K, CCOLS], F16, tag="wb")
            x8 = big.tile([K, NXC], I8, tag="x8")
            XT = big.tile([K, NXC], F16, tag="xt")
            y1t = big.tile([K, NYB], F16, tag="y1t")
            y2t = big.tile([K, NBLK * K], F16, tag="y2t")

            GG = wb[:, 0:GCOLS]
            SEL = wb[:, GCOLS:GCOLS + K]
            IDT = wb[:, GCOLS + K:GCOLS + 2 * K]
            nc.sync.dma_start(out=wb[:, :], in_=win[:, :])
            QC = (CLIPS // 4) * CA
            for c in range(4):       # first quarter per-clip: compute starts sooner
                nc.sync.dma_start(
                    out=x8[:, c * CA:(c + 1) * CA],
                    in_=xin8[:, c * CA:(c + 1) * CA])
            for q in range(1, 4):
                nc.sync.dma_start(
                    out=x8[:, q * QC:(q + 1) * QC],
                    in_=xin8[:, q * QC:(q + 1) * QC])

            # lane observer for the weights DMA: one [K,1] matmul waiting on
            # that DMA's sem lane, so later PE instructions (in-order) never
            # re-wait on it (walrus rejects >1 sync wait per instruction).
            # The int8 loads are consumed by the scalar-engine converts below,
            # and PE waits on those via the scalar lane.
            obs = pbp.tile([K, 1], F32, tag="pb")
            nc.tensor.matmul(obs[:, :], IDT, SEL[:, 0:1], start=True, stop=True)
            # scalar-engine consumer of obs: later writers of this PSUM bank
            # then sync via the Act lane (merged with their existing Act wait)
            # instead of needing a second PE-sem wait (walrus 1-wait limit).
            nc.scalar.mul(y2t[:, NBLK * K - 1:NBLK * K], obs[:, :], 0.0)

            def gf(d):
                return GG[:, d * K:(d + 1) * K]

            def gb(d):
                return GG[:, (D + d) * K:(D + d + 1) * K]


            # forward pass + per-clip constant fill of y1 tail
            for bcl in range(CLIPS):
                xb = bcl * CA
                yb = bcl * CB
                # dequant int8 -> fp16 for this clip (scale folded into gf)
                nc.scalar.mul(XT[:, xb:xb + CA], x8[:, xb:xb + CA], 1.0)
                ps_last = None
                for c0, w in jobs:
                    ps = psp.tile([K, 512], F32, tag="ps")
                    for d in range(D):
                        s0 = xb + c0 + (D - 1) - d
                        nc.tensor.matmul(ps[:, :w], gf(d), XT[:, s0:s0 + w],
                                         start=(d == 0), stop=(d == D - 1))
                    nc.scalar.mul(y1t[:, yb + c0:yb + c0 + w], ps[:, :w], 1.0 / SCALE)
                    ps_last = (ps, w)
                pb = pbp.tile([K, 1], F32, tag="pb")
                nc.tensor.matmul(pb[:, :], SEL, y1t[:, yb + 1250:yb + 1251],
                                 start=True, stop=True)
                for c in range(NYC, CB):
                    nc.scalar.mul(y1t[:, yb + c:yb + c + 1], pb[:, :], 1.0)
                ps3, w3 = ps_last
                nc.scalar.mul(y1t[:, yb + 1250:yb + 1251], pb[:, :], 1.0)
                nc.scalar.mul(y1t[0:102, yb + 1250:yb + 1251],
                              ps3[0:102, w3 - 1:w3], 1.0 / SCALE)

            # backward pass; store each 4-clip quarter of y2t (transposed
            # layout) as soon as it completes — the host un-transposes
            for bcl in range(CLIPS):
                yb = bcl * CB
                zb = bcl * NYC
                for c0, w in jobs:
                    ps = psp.tile([K, 512], F32, tag="ps")
                    for d in range(D):
                        s0 = yb + c0 + d
                        nc.tensor.matmul(ps[:, :w], gb(d), y1t[:, s0:s0 + w],
                                         start=(d == 0), stop=(d == D - 1))
                    nc.scalar.mul(y2t[:, zb + c0:zb + c0 + w], ps[:, :w], 1.0 / SCALE)
                if bcl < 12:
                    if bcl % 4 == 3:
                        q = bcl // 4
                        nc.gpsimd.dma_start(
                            out=yout[:, BOUNDS[q]:BOUNDS[q + 1]],
                            in_=y2t[:, BOUNDS[q]:BOUNDS[q + 1]])
                else:
                    # last quarter: per-clip stores to shrink the kernel tail
                    lo = BOUNDS[3] + (bcl - 12) * NYC
                    nc.gpsimd.dma_start(
                        out=yout[:, lo:lo + NYC],
                        in_=y2t[:, lo:lo + NYC])


    return nc


_NC_CACHE = None


def _enable_jax_compile_cache():
    # persistent XLA compilation cache: run_bass_via_pjrt re-jits a fresh
    # closure every call, so without this each call pays the full XLA
    # compile + neuronx hook (~0.4s). With it, the 2nd call is a disk hit.
    import jax

    try:
        if jax.config.jax_compilation_cache_dir is None:
            jax.config.update("jax_compilation_cache_dir", "/tmp/jax_cc_cache")
            jax.config.update("jax_persistent_cache_min_compile_time_secs", 0)
            jax.config.update("jax_persistent_cache_min_entry_size_bytes", -1)
    except Exception:
        pass


def kernel(audio, b=None, a=None, _want_results_obj=False, _trace=False):
    global _NC_CACHE
    _enable_jax_compile_cache()
    from concourse.bass_utils import run_bass_kernel_spmd

    audio = np.asarray(audio)
    B = audio.shape[0]
    assert audio.shape == (128, T), audio.shape
    if b is None or a is None:
        b, a = _butter_bandpass(ORDER, 2 * LOWER / FS, 2 * UPPER / FS)
    b = np.asarray(b, np.float64)
    a = np.asarray(a, np.float64)

    # dynamic int8 input quantization: scale from the audio amax; the odd
    # extensions / constant pads may exceed it and get clipped — they only
    # set filter warm-up state behind >=640 pre-roll samples, so the clip
    # error decays to nothing before any kept output sample.
    amax = float(max(audio.max(), -audio.min()))
    s = amax / 127.0
    inv = 127.0 / amax
    consts = _build_weights(b, a, s)                 # [128, CCOLS] fp16

    left = 2.0 * audio[:, :1] - audio[:, 1:PAD + 1][:, ::-1]     # f32
    right = 2.0 * audio[:, -1:] - audio[:, -PAD - 1:-1][:, ::-1]
    lq = np.clip(np.rint(left * inv), -127, 127)
    rq = np.clip(np.rint(right * inv), -127, 127)
    A = np.empty((B, CA * K), np.int8)
    A[:, :PL] = lq[:, :1]                            # const ext[0] == left[0]
    A[:, PL:PL + PAD] = lq
    A[:, PL + PAD:PL + PAD + T] = np.rint(audio * inv)
    A[:, PL + PAD + T:PL + TEXT] = rq
    A[:, PL + TEXT:] = rq[:, -1:]

    n_cores = 8
    per = B // n_cores
    in_maps = []
    A3 = A.reshape(B, CA, K)
    for c in range(n_cores):
        xin8 = np.empty((K, NXC), np.int8)
        # [per, CA, K] -> [K, per, CA] single-pass strided copy
        xin8.reshape(K, per, CA)[...] = A3[c * per:(c + 1) * per].transpose(2, 0, 1)
        in_maps.append({"win": consts, "xin8": xin8})

    if _NC_CACHE is None:
        _NC_CACHE = _build_bass()
    import time as _time
    _t0 = _time.time()
    res = run_bass_kernel_spmd(_NC_CACHE, in_maps, core_ids=list(range(n_cores)),
                               trace=_trace)
    res.run_wall_s = _time.time() - _t0

    out = np.empty((B, T), np.float32)
    buf = np.empty((per, NYC * K), np.float32)
    for c in range(n_cores):
        yc3 = res.results[c]["y"].reshape(K, per, NYC)
        # [K, per, NYC] -> [per, NYC, K] single-pass strided copy + f16->f32
        buf.reshape(per, NYC, K)[...] = yc3.transpose(1, 2, 0)
        out[c * per:(c + 1) * per] = buf[:, PAD:PAD + T]
    if _want_results_obj:
        return out, res
    return out


if __name__ == "__main__":
    rng = np.random.default_rng(0)
    audio = rng.standard_normal((128, T)).astype(np.float32)
    y = kernel(audio)
    print("ran:", y.shape, y.dtype, float(np.abs(y).max()))



# revision 21
# speedup vs baseline: 2.3842x; 1.3014x over previous
"""Butterworth bandpass filtfilt on Trainium2 (8 NeuronCores).

Algorithm: the order-16 IIR filtfilt is numerically equivalent (to ~1e-6 rel)
to a truncated-FIR convolution because the slowest pole has radius 0.9808
(impulse response < 1e-7 after ~830 samples).  Each direction becomes 6
PSUM-accumulated block-Toeplitz [128x128] matmuls per 128-sample chunk:
  y1[c] = sum_d G_d @ x[c-d]   (forward,  G_d[j,m] = h[128d + j - m])
  y2[c] = sum_d G_d^T @ y1[c+d] (backward)
with scipy-filtfilt edge handling (odd extension + lfilter_zi constant
extension) folded into constant left/right padding and a per-clip
broadcast fill of y1's last value.

Data layout: batch is sharded 16 clips/core.  The host pre-transposes the
input to [pos-in-chunk, chunk] (partition-major) fp16 and un-transposes the
output (both pure layout permutations); taps are scaled by 4096 to stay in
fp16 normal range and descaled in the PSUM->SBUF copies.  Walrus in this
toolchain allows only ONE semaphore wait per DMA/compute instruction and
~3 on the tail Drain, which dictates: 8 sync-lane input DMAs + 4 SWDGE
output DMAs, "lane observer" matmuls so later PE instructions never need a
second wait, and the _drain_and_barrier split patch below.  The last
output quarter is stored per-clip to shrink the kernel tail.  Cost-model
makespan (TimelineSim): ~83.7us/core vs ~57us memory roofline.
"""

import numpy as np

K = 128
D = 4
SCALE = 4096.0
PAD = 51
T = 160000
TEXT = T + 2 * PAD            # 160102
PL = (D - 1) * K              # 640 constant left pad
CLIPS = 16                    # per core
CA = 1264                     # input chunks per clip (mult of 16; CA*128 >= PL+TEXT)
NYC = 1251                    # valid output chunks per clip
CB = NYC + (D - 1)            # y1 chunks per clip incl const tail
NXC = CLIPS * CA
NYB = CLIPS * CB
NOUT = CLIPS * NYC            # 20016
NBLK = (NOUT + K - 1) // K    # 157
GCOLS = 2 * D * K
CCOLS = GCOLS + 2 * K         # weights + sel + ident
XIN_COLS = CCOLS + NXC
HALF = NOUT // 2              # 10008 column pairs
YP = 3 * HALF                 # 30024 packed 12-bit output bytes per partition
QDEN = 1024.0                 # output quant: y/qs in [-2047, 2047], qs = amax/QDEN

ORDER = 8
FS = 16000.0
LOWER = 300.0
UPPER = 3000.0


def _butter_bandpass(order, w1, w2):
    fs = 2.0
    warped = 2.0 * fs * np.tan(np.pi * np.array([w1, w2]) / fs)
    bw = warped[1] - warped[0]
    wo = np.sqrt(warped[0] * warped[1])
    k = np.arange(1, order + 1)
    p = np.exp(1j * np.pi * (2 * k + order - 1) / (2 * order))
    p_lp = p * (bw / 2.0)
    disc = np.sqrt(p_lp ** 2 - wo ** 2)
    p_bp = np.concatenate([p_lp + disc, p_lp - disc])
    z_bp = np.zeros(order, dtype=complex)
    k_bp = bw ** order
    fs2 = 2.0 * fs
    z_z = np.concatenate([(fs2 + z_bp) / (fs2 - z_bp), -np.ones(order)])
    p_z = (fs2 + p_bp) / (fs2 - p_bp)
    k_z = k_bp * np.real(np.prod(fs2 - z_bp) / np.prod(fs2 - p_bp))
    return np.real(k_z * np.poly(z_z)), np.real(np.poly(p_z))


def _impulse_response(b, a, L):
    n = len(a)
    z = np.zeros(n - 1)
    h = np.zeros(L)
    for t in range(L):
        xt = 1.0 if t == 0 else 0.0
        yt = b[0] * xt + z[0]
        z[:-1] = z[1:]
        z[-1] = 0.0
        z += b[1:] * xt - a[1:] * yt
        h[t] = yt
    return h


def _build_weights(b, a):
    """Static weights. The input int8 scale s=amax/127 and the output 12-bit
    scale qs=amax/QDEN cancel to the data-independent factor QDEN/127 folded
    into the forward block: y1t and the backward PSUM are in qs units."""
    h = _impulse_response(np.asarray(b, np.float64), np.asarray(a, np.float64), D * K + K)
    gf = []  # lhsT for forward: gf_d[m, j] = G_d[j, m] = h[dK + j - m]
    gb = []  # lhsT for backward: gb_d[m, j] = G_d[m, j] = h[dK + m - j]
    hh = np.zeros(D * K + K)
    hh[:len(h)] = h
    mm = np.arange(K)[:, None]
    jj = np.arange(K)[None, :]
    for d in range(D):
        tf = d * K + jj - mm
        tb = d * K + mm - jj
        Gf = np.where((tf >= 0) & (tf < len(hh)), hh[np.clip(tf, 0, len(hh) - 1)], 0.0)
        Gb = np.where((tb >= 0) & (tb < len(hh)), hh[np.clip(tb, 0, len(hh) - 1)], 0.0)
        gf.append(Gf * (QDEN / 127.0))
        gb.append(Gb)
    gpack = np.concatenate(gf + gb, axis=1) * SCALE
    sel = np.zeros((K, K))
    sel[101, :] = 1.0
    ident = np.eye(K)
    return np.concatenate([gpack, sel, ident], axis=1).astype(np.float16)  # [128, CCOLS]


def _build_bass():
    import concourse.bass as bass
    import concourse.mybir as mybir
    from concourse.tile import TileContext
    import concourse.tile as tile_mod
    from concourse.vector_clock import ScopedClock, VectorClock

    # walrus in this toolchain rejects instructions with >~3 sync waits; the
    # Tile tail drain waits on every proc lane in one instruction.  Split it
    # into single-wait drains.
    def _split_drain_and_barrier(self, tick_clock, wait_clock):
        gv = tick_clock.global_clock
        for i, t in enumerate(list(gv)):
            if t <= 0:
                continue
            sub = VectorClock()
            sub.require_at_least(i, t)
            d = self.nc.sync.drain()
            wait_clock.add_sem_waits(d.ins, ScopedClock({None: sub}))
        self.nc.all_engine_barrier()
        assert self.sems is not None
        popped = self.nc._tile_sem_poison_stack.pop()
        assert popped is self._sem_poison
        self.nc.clear_and_free_semaphores(list(self.sems.allocated().values()))
        self.nc.all_engine_barrier()

    tile_mod.TileContext._drain_and_barrier = _split_drain_and_barrier

    F16 = mybir.dt.float16
    F32 = mybir.dt.float32

    I8 = mybir.dt.int8
    I16 = mybir.dt.int16
    AF = mybir.ActivationFunctionType
    AL = mybir.AluOpType

    nc = bass.Bass()
    # const AP for the Identity-activation bias (+2048 12-bit zero offset)
    _ct = nc.alloc_sbuf_tensor("const-float32-2048", [K, 1], F32)
    nc.gpsimd.memset(_ct.ap(), 2048.0)
    nc.const_aps.aps[(F32, 2048.0)] = _ct.ap()
    nc.all_engine_barrier()

    win = nc.dram_tensor("win", [K, CCOLS], F16, kind="ExternalInput")
    xin8 = nc.dram_tensor("xin8", [K, NXC], I8, kind="ExternalInput")
    yout = nc.dram_tensor("y", [K, YP], I8, kind="ExternalOutput")

    jobs = [(0, 512), (512, 512), (1024, NYC - 1024)]

    with TileContext(nc) as tc:
        with (
            tc.tile_pool(name="big", bufs=1) as big,
            tc.tile_pool(name="pk", bufs=1) as pk,
            tc.tile_pool(name="ps", bufs=7, space="PSUM") as psp,
            tc.tile_pool(name="pb", bufs=1, space="PSUM") as pbp,
        ):
            wb = big.tile([K, CCOLS], F16, tag="wb")
            x8 = big.tile([K, NXC], I8, tag="x8")
            XT = big.tile([K, NXC], F16, tag="xt")
            y1t = big.tile([K, NYB], F16, tag="y1t")
            y2t = big.tile([K, NOUT + 8], I16, tag="y2t")   # +8: obs-consumer scratch
            y2p = big.tile([K, YP], I8, tag="y2p")

            GG = wb[:, 0:GCOLS]
            SEL = wb[:, GCOLS:GCOLS + K]
            IDT = wb[:, GCOLS + K:GCOLS + 2 * K]
            nc.sync.dma_start(out=wb[:, :], in_=win[:, :])
            QC = (CLIPS // 4) * CA
            for c in range(4):       # first quarter per-clip: compute starts sooner
                nc.sync.dma_start(
                    out=x8[:, c * CA:(c + 1) * CA],
                    in_=xin8[:, c * CA:(c + 1) * CA])
            for q in range(1, 4):
                nc.sync.dma_start(
                    out=x8[:, q * QC:(q + 1) * QC],
                    in_=xin8[:, q * QC:(q + 1) * QC])

            # lane observer for the weights DMA: one [K,1] matmul waiting on
            # that DMA's sem lane, so later PE instructions (in-order) never
            # re-wait on it (walrus rejects >1 sync wait per instruction).
            # The int8 loads are consumed by the scalar-engine converts below,
            # and PE waits on those via the scalar lane.
            obs = pbp.tile([K, 1], F32, tag="pb")
            nc.tensor.matmul(obs[:, :], IDT, SEL[:, 0:1], start=True, stop=True)
            # scalar-engine consumer of obs: later writers of this PSUM bank
            # then sync via the Act lane (merged with their existing Act wait)
            # instead of needing a second PE-sem wait (walrus 1-wait limit).
            nc.scalar.mul(y2t[:, NOUT:NOUT + 1], obs[:, :], 0.0)

            def gf(d):
                return GG[:, d * K:(d + 1) * K]

            def gb(d):
                return GG[:, (D + d) * K:(D + d + 1) * K]


            # forward pass + per-clip constant fill of y1 tail
            for bcl in range(CLIPS):
                xb = bcl * CA
                yb = bcl * CB
                # dequant int8 -> fp16 for this clip (scale folded into gf)
                nc.scalar.mul(XT[:, xb:xb + CA], x8[:, xb:xb + CA], 1.0)
                ps_last = None
                for c0, w in jobs:
                    ps = psp.tile([K, 512], F32, tag="ps")
                    for d in range(D):
                        s0 = xb + c0 + (D - 1) - d
                        nc.tensor.matmul(ps[:, :w], gf(d), XT[:, s0:s0 + w],
                                         start=(d == 0), stop=(d == D - 1))
                    nc.scalar.mul(y1t[:, yb + c0:yb + c0 + w], ps[:, :w], 1.0 / SCALE)
                    ps_last = (ps, w)
                pb = pbp.tile([K, 1], F32, tag="pb")
                nc.tensor.matmul(pb[:, :], SEL, y1t[:, yb + 1250:yb + 1251],
                                 start=True, stop=True)
                for c in range(NYC, CB):
                    nc.scalar.mul(y1t[:, yb + c:yb + c + 1], pb[:, :], 1.0)
                ps3, w3 = ps_last
                nc.scalar.mul(y1t[:, yb + 1250:yb + 1251], pb[:, :], 1.0)
                nc.scalar.mul(y1t[0:102, yb + 1250:yb + 1251],
                              ps3[0:102, w3 - 1:w3], 1.0 / SCALE)

            # backward pass: PSUM is SCALE * y2/qs; Identity act adds the
            # 12-bit zero offset so y2t holds round(y2/qs) + 2048 in [0, 4095]
            for bcl in range(CLIPS):
                yb = bcl * CB
                zb = bcl * NYC
                for c0, w in jobs:
                    ps = psp.tile([K, 512], F32, tag="ps")
                    for d in range(D):
                        s0 = yb + c0 + d
                        nc.tensor.matmul(ps[:, :w], gb(d), y1t[:, s0:s0 + w],
                                         start=(d == 0), stop=(d == D - 1))
                    nc.scalar.activation(y2t[:, zb + c0:zb + c0 + w], ps[:, :w],
                                         AF.Identity, bias=2048.0, scale=1.0 / SCALE)

            # pack pairs of u12 columns into 3 byte-planes of y2p:
            #   p0 = lo8(a), p2 = b>>4, p1 = hi4(a) | lo4(b)<<4   (each -128
            # for the int8 cast; the host xors 128 back)
            gate = big.tile([K, (NOUT + 1023) // 1024], I16, tag="gate")
            for bi, E in enumerate(range(0, NOUT, 1024)):
                w = min(1024, NOUT - E)
                h = w // 2
                ua = pk.tile([K, 512], I16, tag="ua")
                ub = pk.tile([K, 512], I16, tag="ub")
                tp = pk.tile([K, 512], I16, tag="tp")
                tq = pk.tile([K, 512], I16, tag="tq")
                # gate: sole carrier of this block's Act data wait, written to
                # a fresh column (no reuse hazard -> exactly one sem wait);
                # later DVE ops inherit it via engine order, leaving their
                # pool-reuse waits on the single DVE lane (walrus 1-wait limit)
                nc.vector.tensor_scalar_add(gate[:, bi:bi + 1],
                                            y2t[:, E + w - 1:E + w], 0)
                nc.vector.tensor_scalar(ua[:, :h], y2t[:, E:E + w:2], 4095, 0,
                                        AL.min, AL.max)
                nc.vector.tensor_scalar(ub[:, :h], y2t[:, E + 1:E + w:2], 4095, 0,
                                        AL.min, AL.max)
                o = E // 2
                nc.vector.tensor_scalar(tp[:, :h], ua[:, :h], 255, None, AL.bitwise_and)
                nc.vector.tensor_scalar_add(y2p[:, o:o + h], tp[:, :h], -128)
                nc.vector.tensor_scalar(tq[:, :h], ub[:, :h], 4, None,
                                        AL.logical_shift_right)
                nc.vector.tensor_scalar_add(y2p[:, HALF + o:HALF + o + h], tq[:, :h], -128)
                nc.vector.tensor_scalar(tp[:, :h], ua[:, :h], 8, None,
                                        AL.logical_shift_right)
                nc.vector.tensor_scalar(tq[:, :h], ub[:, :h], 15, 4,
                                        AL.bitwise_and, AL.logical_shift_left)
                nc.vector.tensor_tensor(out=tp[:, :h], in0=tp[:, :h], in1=tq[:, :h],
                                        op=AL.bitwise_or)
                nc.vector.tensor_scalar_add(y2p[:, 2 * HALF + o:2 * HALF + o + h],
                                            tp[:, :h], -128)

            nc.gpsimd.dma_start(out=yout[:, :], in_=y2p[:, :])


    return nc


_NC_CACHE = None


def _enable_jax_compile_cache():
    # persistent XLA compilation cache: run_bass_via_pjrt re-jits a fresh
    # closure every call, so without this each call pays the full XLA
    # compile + neuronx hook (~0.4s). With it, the 2nd call is a disk hit.
    import jax

    try:
        if jax.config.jax_compilation_cache_dir is None:
            jax.config.update("jax_compilation_cache_dir", "/tmp/jax_cc_cache")
            jax.config.update("jax_persistent_cache_min_compile_time_secs", 0)
            jax.config.update("jax_persistent_cache_min_entry_size_bytes", -1)
    except Exception:
        pass


def kernel(audio, b=None, a=None, _want_results_obj=False, _trace=False):
    global _NC_CACHE
    _enable_jax_compile_cache()
    from concourse.bass_utils import run_bass_kernel_spmd

    audio = np.asarray(audio)
    B = audio.shape[0]
    assert audio.shape == (128, T), audio.shape
    if b is None or a is None:
        b, a = _butter_bandpass(ORDER, 2 * LOWER / FS, 2 * UPPER / FS)
    b = np.asarray(b, np.float64)
    a = np.asarray(a, np.float64)

    # dynamic int8 input quantization: scale from the audio amax; the odd
    # extensions / constant pads may exceed it and get clipped — they only
    # set filter warm-up state behind >=640 pre-roll samples, so the clip
    # error decays to nothing before any kept output sample.
    amax = float(max(audio.max(), -audio.min()))
    inv = 127.0 / amax
    qs = amax / QDEN                                 # output 12-bit LSB
    consts = _build_weights(b, a)                    # [128, CCOLS] fp16

    left = 2.0 * audio[:, :1] - audio[:, 1:PAD + 1][:, ::-1]     # f32
    right = 2.0 * audio[:, -1:] - audio[:, -PAD - 1:-1][:, ::-1]
    lq = np.clip(np.rint(left * inv), -127, 127)
    rq = np.clip(np.rint(right * inv), -127, 127)
    A = np.empty((B, CA * K), np.int8)
    A[:, :PL] = lq[:, :1]                            # const ext[0] == left[0]
    A[:, PL:PL + PAD] = lq
    A[:, PL + PAD:PL + PAD + T] = np.rint(audio * inv)
    A[:, PL + PAD + T:PL + TEXT] = rq
    A[:, PL + TEXT:] = rq[:, -1:]

    n_cores = 8
    per = B // n_cores
    in_maps = []
    A3 = A.reshape(B, CA, K)
    for c in range(n_cores):
        xin8 = np.empty((K, NXC), np.int8)
        # [per, CA, K] -> [K, per, CA] single-pass strided copy
        xin8.reshape(K, per, CA)[...] = A3[c * per:(c + 1) * per].transpose(2, 0, 1)
        in_maps.append({"win": consts, "xin8": xin8})

    if _NC_CACHE is None:
        _NC_CACHE = _build_bass()
    import time as _time
    _t0 = _time.time()
    res = run_bass_kernel_spmd(_NC_CACHE, in_maps, core_ids=list(range(n_cores)),
                               trace=_trace)
    res.run_wall_s = _time.time() - _t0

    out = np.empty((B, T), np.float32)
    buf = np.empty((per, NYC * K), np.float32)
    y2q = np.empty((K, NOUT), np.float32)
    for c in range(n_cores):
        raw = res.results[c]["y"]                    # int8 [K, YP]
        u8 = raw.view(np.uint8) ^ 128
        p0 = u8[:, 0:HALF]
        p2 = u8[:, HALF:2 * HALF]
        p1 = u8[:, 2 * HALF:3 * HALF]
        a = (p0.astype(np.uint16) | ((p1 & 15).astype(np.uint16) << 8))
        bq = ((p2.astype(np.uint16) << 4) | (p1 >> 4))
        y2q[:, 0::2] = a
        y2q[:, 1::2] = bq
        y2q -= 2048.0
        # [K, per, NYC] -> [per, NYC, K] single-pass strided copy
        buf.reshape(per, NYC, K)[...] = y2q.reshape(K, per, NYC).transpose(1, 2, 0)
        out[c * per:(c + 1) * per] = buf[:, PAD:PAD + T]
    out *= qs
    if _want_results_obj:
        return out, res
    return out


if __name__ == "__main__":
    rng = np.random.default_rng(0)
    audio = rng.standard_normal((128, T)).astype(np.float32)
    y = kernel(audio)
    print("ran:", y.shape, y.dtype, float(np.abs(y).max()))

